# revision 4
# baseline (speedup 1.0000x reference)
"""Trainium2 Bass kernel for nn_CombineUV (shortlist-scored retrieval).

Math: out[b,s] = dot(input[b], sig(alpha)*weight[i] + sig(beta)*labels[i]) + bias[i]
with i = shortlist[b,s].  Folding the sigmoid gates into the input side:
out[b,s] = dot(xa[b], weight[i]) + dot(xb[b], labels[i]) + bias[i]
where xa = input*sig(alpha), xb = input*sig(beta) -- the [L,D] combined
table is never materialized and no arithmetic on table values happens on host.

Device strategy (8 cores, L-sharded, all-stream dedup + window merging):
 - Combined table TC = [weight || labels] as [L, 1024] bf16; core c owns rows
   [c*16384, (c+1)*16384).  Every (b,s) pair is routed to the core owning its
   row.  The host pre-transposes ONE stream column per distinct (row, spill)
   into PE-ready [128, 8*W] tiles that load with plain full-rate dma_start --
   there is NO dma_gather path at all (the old SWDGE descriptor-gen chain was
   the baseline's bottleneck: ~5us of serial Q7 work per 512 rows).
 - Batch axis is split into 4 quarters of 128.  A streamed column serves ALL
   its pairs: for each quarter its row is hit in, the tile gets one extra
   "window pass" (8 accumulating matmuls with the xc slice of that quarter's
   128 batches) over the SAME streamed data -- extra PE work, zero extra DMA.
   Columns are grouped into tiles by their exact quarter-hit-set so no window
   pass is wasted.  Per window up to 2 pairs/column are extracted by mask
   passes (host-built one-hot over the 128 window rows, multiplied on DVE,
   reduced to a row via a ones-vector matmul); >2 pairs per (row, quarter)
   spill to an extra column instance.
 - Host adds bias[shortlist] (O(B*S) elementwise) and inverse-permutes.
"""

import sys

sys.path.insert(0, "/opt/trn_rl_repo")

import numpy as np
import ml_dtypes

BF16 = ml_dtypes.bfloat16

L, D, B, S = 131072, 512, 512, 512
NCORES = 8
LSH = L // NCORES          # table rows per core
NCHUNK = 8                 # combined-row chunks of 128 (2*D = 1024 bf16)
TILE = 512                 # max columns per streamed tile
QW = 128                   # quarter window width
NQ = B // QW               # 4 quarters
WGRAN = 32                 # tail-tile width granularity
CH = 8                     # mask/out passes per DMA chunk

_PROG_CACHE = {}


def _emit_columns(lidx, bvec, pos):
    """Group one core's pairs into column instances.

    Returns a list of columns; each column is (row, {q: [(m, flatpos), ...]})
    with at most 2 pairs per quarter q (m = b - q*128).  Rows with >2 pairs in
    a quarter emit extra column instances (spills).
    """
    order = np.lexsort((bvec, lidx))
    li, bv, ps = lidx[order], bvec[order], pos[order]
    cols = []
    n = len(li)
    i = 0
    while i < n:
        j = i
        while j < n and li[j] == li[i]:
            j += 1
        # pairs of this row, already b-sorted
        byq = {}
        for k in range(i, j):
            q = int(bv[k]) // QW
            byq.setdefault(q, []).append((int(bv[k]) - q * QW, int(ps[k])))
        inst = 0
        while byq:
            served = {}
            for q in list(byq):
                served[q] = byq[q][:2]
                del byq[q][:2]
                if not byq[q]:
                    del byq[q]
            cols.append((int(li[i]), served))
            inst += 1
        i = j
    return cols


def _build_structure(cols_by_core):
    """Unify per-core column lists into one shared program structure.

    Returns (tiles, total_w8, npass_total, percore) where tiles is a list of
    dicts {type, w (padded width), wreal, st_off (in elements/8), windows:
    [(q, npass)], slot0}, and percore[c] holds (cols list aligned to tiles).
    """
    # collect all types
    typeset = set()
    for cols in cols_by_core:
        for _, served in cols:
            typeset.add(tuple(sorted(served)))
    types = sorted(typeset, key=lambda t: (len(t), t))

    # per core per type: sorted column lists (2nd-pair-rich first)
    percore_by_type = []
    for cols in cols_by_core:
        byt = {t: [] for t in types}
        for row, served in cols:
            byt[tuple(sorted(served))].append((row, served))
        for t in types:
            byt[t].sort(key=lambda rc: -sum(len(v) > 1 for v in rc[1].values()))
        percore_by_type.append(byt)

    tiles = []
    st_off = 0
    for t in types:
        n_t = max(len(pc[t]) for pc in percore_by_type)
        done = 0
        while done < n_t:
            wreal = min(TILE, n_t - done)
            w = max(WGRAN, -(-wreal // WGRAN) * WGRAN)
            tiles.append(
                {
                    "type": t,
                    "w": w,
                    "wreal": wreal,
                    "off": done,
                    "st_off": st_off,
                    "idx_in_type": len([x for x in tiles if x["type"] == t]),
                }
            )
            st_off += NCHUNK * w
            done += wreal
    # interleave types for an even engine mix
    tiles.sort(key=lambda x: (x["idx_in_type"], types.index(x["type"])))

    # per (tile, window): npass = max over cores of per-column served count
    slot = 0
    for tl in tiles:
        t, off, wreal = tl["type"], tl["off"], tl["wreal"]
        windows = []
        for q in t:
            npass = 1
            for pc in percore_by_type:
                lst = pc[t][off : off + wreal]
                for _, served in lst:
                    if len(served.get(q, ())) > 1:
                        npass = 2
                        break
                if npass == 2:
                    break
            windows.append((q, npass))
            slot += npass
        tl["windows"] = windows
    # assign slots
    slot = 0
    for tl in tiles:
        tl["slot0"] = slot
        slot += sum(np_ for _, np_ in tl["windows"])
    return tiles, st_off, slot, percore_by_type, types


def _build_program(sig, tiles, total_w8, npass):
    import concourse.bacc as bacc
    import concourse.mybir as mybir
    from concourse.tile import TileContext

    f32, bf, u8 = mybir.dt.float32, mybir.dt.bfloat16, mybir.dt.uint8
    nmch = -(-npass // CH)

    nc = bacc.Bacc(None, target_bir_lowering=False)
    st_d = nc.dram_tensor("st", [128, total_w8], bf, kind="ExternalInput")
    xc_d = nc.dram_tensor("xc", [128, NCHUNK * B], bf, kind="ExternalInput")
    mask_d = nc.dram_tensor("mask", [nmch, 128, CH * TILE], u8, kind="ExternalInput")
    ones_d = nc.dram_tensor("ones", [128, 1], bf, kind="ExternalInput")
    out_d = nc.dram_tensor("out", [nmch, CH * TILE], f32, kind="ExternalOutput")

    with TileContext(nc) as tc:
        with (
            tc.tile_pool(name="res", bufs=1) as res_pool,
            tc.tile_pool(name="g", bufs=5) as gpool,
            tc.tile_pool(name="mc", bufs=4) as mcpool,
            tc.tile_pool(name="m", bufs=6) as mpool,
            tc.tile_pool(name="acc", bufs=3) as accpool,
            tc.tile_pool(name="ps", bufs=4, space="PSUM") as pspool,
            tc.tile_pool(name="ps2", bufs=3, space="PSUM") as ps2pool,
        ):
            xc_sb = res_pool.tile([128, NCHUNK * B], bf, tag="xc")
            nc.sync.dma_start(out=xc_sb[:], in_=xc_d[:])
            ones_sb = res_pool.tile([128, 1], bf, tag="ones")
            nc.sync.dma_start(out=ones_sb[:], in_=ones_d[:])

            nmch_total = -(-npass // CH)
            mc_tiles = {}
            acc_state = {"cur": None}

            def ensure_mc(k):
                if k < nmch_total and k not in mc_tiles:
                    mct = mcpool.tile([128, CH * TILE], u8, tag="mc")
                    nc.sync.dma_start(out=mct[:], in_=mask_d[k])
                    mc_tiles[k] = mct

            def select_stage(ps, slot0, w, np_):
                for p in range(np_):
                    slot = slot0 + p
                    mch, moff = slot // CH, (slot % CH) * TILE
                    ensure_mc(mch)
                    ensure_mc(mch + 1)
                    cur_acc = acc_state["cur"]
                    if cur_acc is None or mch != cur_acc[0]:
                        if cur_acc is not None:
                            nc.sync.dma_start(
                                out=out_d[cur_acc[0] : cur_acc[0] + 1, :],
                                in_=cur_acc[1][:],
                            )
                        acct = accpool.tile([1, CH * TILE], f32, tag="acc")
                        cur_acc = (mch, acct)
                        acc_state["cur"] = cur_acc
                    msk = mpool.tile([128, TILE], bf, tag="msk")
                    nc.vector.tensor_tensor(
                        out=msk[:, :w],
                        in0=ps[:, :w],
                        in1=mc_tiles[mch][:, moff : moff + w],
                        op=mybir.AluOpType.mult,
                    )
                    ps2 = ps2pool.tile([1, TILE], f32, tag="ps2")
                    nc.tensor.matmul(
                        out=ps2[:, :w],
                        lhsT=ones_sb[:],
                        rhs=msk[:, :w],
                        start=True,
                        stop=True,
                    )
                    nc.scalar.copy(cur_acc[1][:, moff : moff + w], ps2[:, :w])

            # The select stage (DVE mask-mult -> PE ones-reduce -> ACT copy) of
            # window i is emitted AFTER window i+1's chunk matmuls: the reduce
            # matmul sits in the PE FIFO behind work that hides the DVE
            # latency, instead of stalling the PE every window.
            pending = None
            for tl in tiles:
                w = tl["w"]
                g = gpool.tile([128, NCHUNK * TILE], bf, tag="g")
                nc.sync.dma_start(
                    out=g[:, : NCHUNK * w],
                    in_=st_d[:, tl["st_off"] : tl["st_off"] + NCHUNK * w],
                )
                slot = tl["slot0"]
                for q, np_ in tl["windows"]:
                    ps = pspool.tile([128, TILE], f32, tag="ps")
                    for c in range(NCHUNK):
                        nc.tensor.matmul(
                            out=ps[:, :w],
                            lhsT=xc_sb[:, c * B + q * QW : c * B + q * QW + QW],
                            rhs=g[:, c * w : (c + 1) * w],
                            start=(c == 0),
                            stop=(c == NCHUNK - 1),
                        )
                    if pending is not None:
                        select_stage(*pending)
                    pending = (ps, slot, w, np_)
                    slot += np_
            if pending is not None:
                select_stage(*pending)
            cur_acc = acc_state["cur"]
            if cur_acc is not None:
                nc.sync.dma_start(
                    out=out_d[cur_acc[0] : cur_acc[0] + 1, :], in_=cur_acc[1][:]
                )

    nc.compile()
    return nc


def _prep_inputs(input, labels, weight, alpha, beta, shortlist):
    input = np.asarray(input, dtype=np.float32)
    alpha = np.asarray(alpha, dtype=np.float32).reshape(1, D)
    beta = np.asarray(beta, dtype=np.float32).reshape(1, D)
    xa = input * (1.0 / (1.0 + np.exp(-alpha)))
    xb = input * (1.0 / (1.0 + np.exp(-beta)))

    # XC[p, c, b]: chunk c of xa (c<4) / xb (c>=4) for batch b.
    XC = np.empty((128, NCHUNK, B), dtype=BF16)
    XC[:, : NCHUNK // 2, :] = xa.T.reshape(NCHUNK // 2, 128, B).transpose(1, 0, 2)
    XC[:, NCHUNK // 2 :, :] = xb.T.reshape(NCHUNK // 2, 128, B).transpose(1, 0, 2)

    TC = np.concatenate(
        [np.asarray(weight, np.float32), np.asarray(labels, np.float32)], axis=1
    ).astype(BF16)  # [L, 1024]

    sl = np.asarray(shortlist).reshape(-1).astype(np.int64)
    core = sl // LSH
    lidx = sl % LSH
    bvec = np.repeat(np.arange(B, dtype=np.int64), S)
    allpos = np.arange(B * S, dtype=np.int64)

    cols_by_core = []
    for c in range(NCORES):
        m = core == c
        cols_by_core.append(_emit_columns(lidx[m], bvec[m], allpos[m]))

    tiles, total_w8, npass, percore_by_type, types = _build_structure(cols_by_core)
    nmch = -(-npass // CH)

    in_maps = []
    posmaps = []
    ones = np.ones((128, 1), dtype=BF16)
    xc_flat = np.ascontiguousarray(XC.reshape(128, NCHUNK * B))
    for c in range(NCORES):
        st = np.zeros((128, total_w8), dtype=BF16)
        maskh = np.zeros((nmch, 128, CH * TILE), dtype=np.uint8)
        posmap = np.full((nmch * CH, TILE), -1, dtype=np.int64)
        byt = percore_by_type[c]
        for tl in tiles:
            t, off, wreal, w = tl["type"], tl["off"], tl["wreal"], tl["w"]
            lst = byt[t][off : off + wreal]
            if lst:
                rows = np.array([r for r, _ in lst], np.int64)
                # st[p, st_off + ch*w + j] = TC_local[row_j, ch*128 + p]
                arr = TC[c * LSH : (c + 1) * LSH][rows]  # [ncols, 1024]
                arr = arr.reshape(len(rows), NCHUNK, 128)  # [j, ch, p]
                st[:, tl["st_off"] : tl["st_off"] + NCHUNK * w].reshape(
                    128, NCHUNK, w
                )[:, :, : len(rows)] = arr.transpose(2, 1, 0)
            slot = tl["slot0"]
            for q, np_ in tl["windows"]:
                for p in range(np_):
                    mch, moff = slot // CH, (slot % CH) * TILE
                    for j, (_, served) in enumerate(lst):
                        pair = served.get(q, ())
                        if len(pair) > p:
                            mval, fpos = pair[p]
                            maskh[mch, mval, moff + j] = 1
                            posmap[slot, j] = fpos
                    slot += 1
        in_maps.append(
            {
                "st": st,
                "xc": xc_flat,
                "mask": maskh,
                "ones": ones,
            }
        )
        posmaps.append(posmap)

    sig = tuple(
        (tuple(tl["type"]), tl["w"], tuple(tl["windows"])) for tl in tiles
    )
    return sig, tiles, total_w8, npass, in_maps, posmaps


def kernel(input, labels, weight, alpha, beta, bias, shortlist, _trace=False):
    from concourse.bass_utils import run_bass_kernel_spmd

    sig, tiles, total_w8, npass, in_maps, posmaps = _prep_inputs(
        input, labels, weight, alpha, beta, shortlist
    )

    if sig not in _PROG_CACHE:
        _PROG_CACHE[sig] = _build_program(sig, tiles, total_w8, npass)
    nc = _PROG_CACHE[sig]

    res = run_bass_kernel_spmd(nc, in_maps, list(range(NCORES)), trace=_trace)

    out_flat = np.zeros(B * S, dtype=np.float32)
    for c in range(NCORES):
        vals = res.results[c]["out"].reshape(-1, TILE)  # [nmch*CH, TILE]
        pm = posmaps[c]
        sel = pm >= 0
        out_flat[pm[sel]] = vals[: pm.shape[0]][sel]

    bias = np.asarray(bias, dtype=np.float32)
    sl = np.asarray(shortlist).reshape(-1).astype(np.int64)
    out_flat += bias[sl]
    out = out_flat.reshape(B, S)

    if _trace:
        return out, res
    return out


# revision 5
# speedup vs baseline: 1.1161x; 1.1161x over previous
"""Trainium2 Bass kernel for nn_CombineUV (shortlist-scored retrieval).

Math: out[b,s] = dot(input[b], sig(alpha)*weight[i] + sig(beta)*labels[i]) + bias[i]
with i = shortlist[b,s].  Folding the sigmoid gates into the input side:
out[b,s] = dot(xa[b], weight[i]) + dot(xb[b], labels[i]) + bias[i]
where xa = input*sig(alpha), xb = input*sig(beta) -- the [L,D] combined
table is never materialized and no arithmetic on table values happens on host.

Device strategy (8 cores, L-sharded, all-stream dedup + window merging):
 - Combined table TC = [weight || labels] as [L, 1024] bf16; core c owns rows
   [c*16384, (c+1)*16384).  Every (b,s) pair is routed to the core owning its
   row.  The host pre-transposes ONE stream column per distinct (row, spill)
   into PE-ready [128, 8*W] tiles that load with plain full-rate dma_start --
   there is NO dma_gather path at all (the old SWDGE descriptor-gen chain was
   the baseline's bottleneck: ~5us of serial Q7 work per 512 rows).
 - Batch axis is split into 4 quarters of 128.  A streamed column serves ALL
   its pairs: for each quarter its row is hit in, the tile gets one extra
   "window pass" (8 accumulating matmuls with the xc slice of that quarter's
   128 batches) over the SAME streamed data -- extra PE work, zero extra DMA.
   Columns are grouped into tiles by their exact quarter-hit-set so no window
   pass is wasted.  Per window up to 2 pairs/column are extracted by mask
   passes (host-built one-hot over the 128 window rows, multiplied on DVE,
   reduced to a row via a ones-vector matmul); >2 pairs per (row, quarter)
   spill to an extra column instance.
 - Host adds bias[shortlist] (O(B*S) elementwise) and inverse-permutes.
"""

import sys

sys.path.insert(0, "/opt/trn_rl_repo")

import numpy as np
import ml_dtypes

BF16 = ml_dtypes.bfloat16

L, D, B, S = 131072, 512, 512, 512
NCORES = 8
LSH = L // NCORES          # table rows per core
NCHUNK = 8                 # combined-row chunks of 128 (2*D = 1024 bf16)
TILE = 512                 # max columns per streamed tile
QW = 128                   # quarter window width
NQ = B // QW               # 4 quarters
WGRAN = 32                 # tail-tile width granularity
CH = 8                     # mask/out passes per DMA chunk

_PROG_CACHE = {}


def _emit_columns(lidx, bvec, pos):
    """Group one core's pairs into column instances.

    Returns a list of columns; each column is (row, {q: [(m, flatpos), ...]})
    with at most 2 pairs per quarter q (m = b - q*128).  Rows with >2 pairs in
    a quarter emit extra column instances (spills).
    """
    order = np.lexsort((bvec, lidx))
    li, bv, ps = lidx[order], bvec[order], pos[order]
    cols = []
    n = len(li)
    i = 0
    while i < n:
        j = i
        while j < n and li[j] == li[i]:
            j += 1
        # pairs of this row, already b-sorted
        byq = {}
        for k in range(i, j):
            q = int(bv[k]) // QW
            byq.setdefault(q, []).append((int(bv[k]) - q * QW, int(ps[k])))
        inst = 0
        while byq:
            served = {}
            for q in list(byq):
                served[q] = byq[q][:2]
                del byq[q][:2]
                if not byq[q]:
                    del byq[q]
            cols.append((int(li[i]), served))
            inst += 1
        i = j
    return cols


def _build_structure(cols_by_core):
    """Unify per-core column lists into one shared program structure.

    Returns (tiles, total_w8, npass_total, percore) where tiles is a list of
    dicts {type, w (padded width), wreal, st_off (in elements/8), windows:
    [(q, npass)], slot0}, and percore[c] holds (cols list aligned to tiles).
    """
    # collect all types
    typeset = set()
    for cols in cols_by_core:
        for _, served in cols:
            typeset.add(tuple(sorted(served)))
    types = sorted(typeset, key=lambda t: (len(t), t))

    # per core per type: sorted column lists (2nd-pair-rich first)
    percore_by_type = []
    for cols in cols_by_core:
        byt = {t: [] for t in types}
        for row, served in cols:
            byt[tuple(sorted(served))].append((row, served))
        for t in types:
            byt[t].sort(key=lambda rc: -sum(len(v) > 1 for v in rc[1].values()))
        percore_by_type.append(byt)

    tiles = []
    st_off = 0
    for t in types:
        n_t = max(len(pc[t]) for pc in percore_by_type)
        done = 0
        while done < n_t:
            wreal = min(TILE, n_t - done)
            w = max(WGRAN, -(-wreal // WGRAN) * WGRAN)
            tiles.append(
                {
                    "type": t,
                    "w": w,
                    "wreal": wreal,
                    "off": done,
                    "st_off": st_off,
                    "idx_in_type": len([x for x in tiles if x["type"] == t]),
                }
            )
            st_off += NCHUNK * w
            done += wreal
    # interleave types for an even engine mix
    tiles.sort(key=lambda x: (x["idx_in_type"], types.index(x["type"])))

    # per (tile, window): npass = max over cores of per-column served count
    slot = 0
    for tl in tiles:
        t, off, wreal = tl["type"], tl["off"], tl["wreal"]
        windows = []
        for q in t:
            npass = 1
            for pc in percore_by_type:
                lst = pc[t][off : off + wreal]
                for _, served in lst:
                    if len(served.get(q, ())) > 1:
                        npass = 2
                        break
                if npass == 2:
                    break
            windows.append((q, npass))
            slot += npass
        tl["windows"] = windows
    # assign slots
    slot = 0
    for tl in tiles:
        tl["slot0"] = slot
        slot += sum(np_ for _, np_ in tl["windows"])
    return tiles, st_off, slot, percore_by_type, types


def _build_program(sig, tiles, total_w8, npass):
    import concourse.bacc as bacc
    import concourse.mybir as mybir
    from concourse.tile import TileContext

    f32, bf, u8 = mybir.dt.float32, mybir.dt.bfloat16, mybir.dt.uint8
    nmch = -(-npass // CH)

    nc = bacc.Bacc(None, target_bir_lowering=False)
    st_d = nc.dram_tensor("st", [128, total_w8], bf, kind="ExternalInput")
    xc_d = nc.dram_tensor("xc", [128, NCHUNK * B], bf, kind="ExternalInput")
    mask_d = nc.dram_tensor("mask", [nmch, 128, CH * TILE], u8, kind="ExternalInput")
    ones_d = nc.dram_tensor("ones", [128, 1], bf, kind="ExternalInput")
    out_d = nc.dram_tensor("out", [nmch, CH * TILE], f32, kind="ExternalOutput")

    with TileContext(nc) as tc:
        with (
            tc.tile_pool(name="res", bufs=1) as res_pool,
            tc.tile_pool(name="g", bufs=5) as gpool,
            tc.tile_pool(name="mc", bufs=4) as mcpool,
            tc.tile_pool(name="m", bufs=6) as mpool,
            tc.tile_pool(name="acc", bufs=3) as accpool,
            tc.tile_pool(name="ps", bufs=4, space="PSUM") as pspool,
            tc.tile_pool(name="ps2", bufs=3, space="PSUM") as ps2pool,
        ):
            xc_sb = res_pool.tile([128, NCHUNK * B], bf, tag="xc")
            nc.sync.dma_start(out=xc_sb[:], in_=xc_d[:])
            ones_sb = res_pool.tile([128, 1], bf, tag="ones")
            nc.sync.dma_start(out=ones_sb[:], in_=ones_d[:])

            nmch_total = -(-npass // CH)
            mc_tiles = {}
            acc_state = {"cur": None}

            def ensure_mc(k):
                if k < nmch_total and k not in mc_tiles:
                    mct = mcpool.tile([128, CH * TILE], u8, tag="mc")
                    nc.sync.dma_start(out=mct[:], in_=mask_d[k])
                    mc_tiles[k] = mct

            def select_stage(ps, slot0, w, np_):
                for p in range(np_):
                    slot = slot0 + p
                    mch, moff = slot // CH, (slot % CH) * TILE
                    ensure_mc(mch)
                    ensure_mc(mch + 1)
                    cur_acc = acc_state["cur"]
                    if cur_acc is None or mch != cur_acc[0]:
                        if cur_acc is not None:
                            nc.scalar.dma_start(
                                out=out_d[cur_acc[0] : cur_acc[0] + 1, :],
                                in_=cur_acc[1][:],
                            )
                        acct = accpool.tile([1, CH * TILE], f32, tag="acc")
                        cur_acc = (mch, acct)
                        acc_state["cur"] = cur_acc
                    msk = mpool.tile([128, TILE], bf, tag="msk")
                    nc.vector.tensor_tensor(
                        out=msk[:, :w],
                        in0=ps[:, :w],
                        in1=mc_tiles[mch][:, moff : moff + w],
                        op=mybir.AluOpType.mult,
                    )
                    ps2 = ps2pool.tile([1, TILE], f32, tag="ps2")
                    nc.tensor.matmul(
                        out=ps2[:, :w],
                        lhsT=ones_sb[:],
                        rhs=msk[:, :w],
                        start=True,
                        stop=True,
                    )
                    nc.scalar.copy(cur_acc[1][:, moff : moff + w], ps2[:, :w])

            # The select stage (DVE mask-mult -> PE ones-reduce -> ACT copy) of
            # window i is emitted AFTER window i+1's chunk matmuls: the reduce
            # matmul sits in the PE FIFO behind work that hides the DVE
            # latency, instead of stalling the PE every window.
            pending = None
            for tl in tiles:
                w = tl["w"]
                g = gpool.tile([128, NCHUNK * TILE], bf, tag="g")
                nc.sync.dma_start(
                    out=g[:, : NCHUNK * w],
                    in_=st_d[:, tl["st_off"] : tl["st_off"] + NCHUNK * w],
                )
                slot = tl["slot0"]
                for q, np_ in tl["windows"]:
                    ps = pspool.tile([128, TILE], f32, tag="ps")
                    for c in range(NCHUNK):
                        nc.tensor.matmul(
                            out=ps[:, :w],
                            lhsT=xc_sb[:, c * B + q * QW : c * B + q * QW + QW],
                            rhs=g[:, c * w : (c + 1) * w],
                            start=(c == 0),
                            stop=(c == NCHUNK - 1),
                        )
                    if pending is not None:
                        select_stage(*pending)
                    pending = (ps, slot, w, np_)
                    slot += np_
            if pending is not None:
                select_stage(*pending)
            cur_acc = acc_state["cur"]
            if cur_acc is not None:
                nc.scalar.dma_start(
                    out=out_d[cur_acc[0] : cur_acc[0] + 1, :], in_=cur_acc[1][:]
                )

    nc.compile()
    return nc


def _prep_inputs(input, labels, weight, alpha, beta, shortlist):
    input = np.asarray(input, dtype=np.float32)
    alpha = np.asarray(alpha, dtype=np.float32).reshape(1, D)
    beta = np.asarray(beta, dtype=np.float32).reshape(1, D)
    xa = input * (1.0 / (1.0 + np.exp(-alpha)))
    xb = input * (1.0 / (1.0 + np.exp(-beta)))

    # XC[p, c, b]: chunk c of xa (c<4) / xb (c>=4) for batch b.
    XC = np.empty((128, NCHUNK, B), dtype=BF16)
    XC[:, : NCHUNK // 2, :] = xa.T.reshape(NCHUNK // 2, 128, B).transpose(1, 0, 2)
    XC[:, NCHUNK // 2 :, :] = xb.T.reshape(NCHUNK // 2, 128, B).transpose(1, 0, 2)

    TC = np.concatenate(
        [np.asarray(weight, np.float32), np.asarray(labels, np.float32)], axis=1
    ).astype(BF16)  # [L, 1024]

    sl = np.asarray(shortlist).reshape(-1).astype(np.int64)
    core = sl // LSH
    lidx = sl % LSH
    bvec = np.repeat(np.arange(B, dtype=np.int64), S)
    allpos = np.arange(B * S, dtype=np.int64)

    cols_by_core = []
    for c in range(NCORES):
        m = core == c
        cols_by_core.append(_emit_columns(lidx[m], bvec[m], allpos[m]))

    tiles, total_w8, npass, percore_by_type, types = _build_structure(cols_by_core)
    nmch = -(-npass // CH)

    in_maps = []
    posmaps = []
    ones = np.ones((128, 1), dtype=BF16)
    xc_flat = np.ascontiguousarray(XC.reshape(128, NCHUNK * B))
    for c in range(NCORES):
        st = np.zeros((128, total_w8), dtype=BF16)
        maskh = np.zeros((nmch, 128, CH * TILE), dtype=np.uint8)
        posmap = np.full((nmch * CH, TILE), -1, dtype=np.int64)
        byt = percore_by_type[c]
        for tl in tiles:
            t, off, wreal, w = tl["type"], tl["off"], tl["wreal"], tl["w"]
            lst = byt[t][off : off + wreal]
            if lst:
                rows = np.array([r for r, _ in lst], np.int64)
                # st[p, st_off + ch*w + j] = TC_local[row_j, ch*128 + p]
                arr = TC[c * LSH : (c + 1) * LSH][rows]  # [ncols, 1024]
                arr = arr.reshape(len(rows), NCHUNK, 128)  # [j, ch, p]
                st[:, tl["st_off"] : tl["st_off"] + NCHUNK * w].reshape(
                    128, NCHUNK, w
                )[:, :, : len(rows)] = arr.transpose(2, 1, 0)
            slot = tl["slot0"]
            for q, np_ in tl["windows"]:
                for p in range(np_):
                    mch, moff = slot // CH, (slot % CH) * TILE
                    for j, (_, served) in enumerate(lst):
                        pair = served.get(q, ())
                        if len(pair) > p:
                            mval, fpos = pair[p]
                            maskh[mch, mval, moff + j] = 1
                            posmap[slot, j] = fpos
                    slot += 1
        in_maps.append(
            {
                "st": st,
                "xc": xc_flat,
                "mask": maskh,
                "ones": ones,
            }
        )
        posmaps.append(posmap)

    sig = tuple(
        (tuple(tl["type"]), tl["w"], tuple(tl["windows"])) for tl in tiles
    )
    return sig, tiles, total_w8, npass, in_maps, posmaps


def kernel(input, labels, weight, alpha, beta, bias, shortlist, _trace=False):
    from concourse.bass_utils import run_bass_kernel_spmd

    sig, tiles, total_w8, npass, in_maps, posmaps = _prep_inputs(
        input, labels, weight, alpha, beta, shortlist
    )

    if sig not in _PROG_CACHE:
        _PROG_CACHE[sig] = _build_program(sig, tiles, total_w8, npass)
    nc = _PROG_CACHE[sig]

    res = run_bass_kernel_spmd(nc, in_maps, list(range(NCORES)), trace=_trace)

    out_flat = np.zeros(B * S, dtype=np.float32)
    for c in range(NCORES):
        vals = res.results[c]["out"].reshape(-1, TILE)  # [nmch*CH, TILE]
        pm = posmaps[c]
        sel = pm >= 0
        out_flat[pm[sel]] = vals[: pm.shape[0]][sel]

    bias = np.asarray(bias, dtype=np.float32)
    sl = np.asarray(shortlist).reshape(-1).astype(np.int64)
    out_flat += bias[sl]
    out = out_flat.reshape(B, S)

    if _trace:
        return out, res
    return out
